# revision 5
# baseline (speedup 1.0000x reference)
"""Causal single-head attention (B=8, T=2048, E=1024, H=64) on 8 trn2 cores.

Sharding: data-parallel over batch; core b computes batch b end-to-end.

Device algorithm (per core), v2:
  xT [E,T] arrives pre-transposed from host. Superblocks (NQ=512) are
  processed in PAIRS so the k-projection can col-tile two superblocks
  into one PE pass.
  - qvT[128,NQ] per sb: matmul pass with packed stationary [Wq|Wv].
  - k-projection PAIRED: tile (0,0) computes k(sb even) -> psum rows
    0:64, tile (0,64) computes k(sb odd) -> rows 64:128, concurrently
    (two col-groups, two rhs streams). Copies land in kST[128, T/2]:
    even key-tiles on partitions 0:64, odd on 64:128.
  - q is duplicated to partitions 64:128 (qd) so scores can ROW-TILE:
    ST for key-tiles (2kp, 2kp+1) run concurrently as two K=64 matmuls
    on row-groups (0,0) and (64,0) -> 2x score throughput.
  - P = exp(ST/32) on ACT; no row-max needed (|S/32| <= ~0.6).
    Causality: k-tiles above the diagonal are skipped, straddling
    tiles multiply by an upper-triangular 0/1 mask.
  - oT[65,tq] accumulates over k-tiles: lhsT = [v | ones][128,65],
    rhs = P. Row 64 = softmax denominator. Padding mask folded into
    [v | ones] rows.
  - oT 128-col tiles are PE-transposed to [128,65]; out = cols0:64 *
    reciprocal(col 64); DMA to DRAM out[T,64].
"""

import numpy as np

import concourse.bass as bass
import concourse.mybir as mybir
import concourse.tile as tile
from concourse import bacc
from concourse.bass_utils import run_bass_kernel_spmd
from concourse.masks import make_identity, make_upper_triangular

B, T, E, H = 8, 2048, 1024, 64
NQ = 512              # query superblock (columns of ST / oT)
N_QSB = T // NQ       # 4
N_KT = T // 128       # 16 key tiles
N_ET = E // 128       # 8 contraction tiles
SCALE = float(E) ** -0.5

MM_DT = mybir.dt.float16

_CACHE = {}


def _build(repeat=1):
    f32 = mybir.dt.float32
    nc = bacc.Bacc("TRN2", target_bir_lowering=False)
    mmdt = MM_DT
    xT_d = nc.dram_tensor("xT", [E, T], mmdt, kind="ExternalInput")
    wqv_d = nc.dram_tensor("wqv", [E, 128], mmdt, kind="ExternalInput")
    wk_d = nc.dram_tensor("wk", [E, H], mmdt, kind="ExternalInput")
    km_d = nc.dram_tensor("kmask", [T], f32, kind="ExternalInput")
    out_d = nc.dram_tensor("out", [T, H], f32, kind="ExternalOutput")

    with tile.TileContext(nc) as tc:
        with (
            tc.tile_pool(name="consts", bufs=1) as consts,
            tc.tile_pool(name="xt", bufs=4) as xt_pool,
            tc.tile_pool(name="big", bufs=1) as big,
            tc.tile_pool(name="pt", bufs=4) as pt_pool,
            tc.tile_pool(name="otsb", bufs=2) as otsb_pool,
            tc.tile_pool(name="osb", bufs=4) as osb_pool,
            tc.tile_pool(name="small", bufs=4) as small_pool,
            tc.tile_pool(name="ps_proj", bufs=2, space="PSUM") as ps_proj,
            tc.tile_pool(name="ps_st", bufs=2, space="PSUM") as ps_st,
            tc.tile_pool(name="ps_ot", bufs=2, space="PSUM") as ps_ot,
        ):
            # ---- constants ----
            ident16 = consts.tile([128, 128], mmdt)
            make_identity(nc, ident16)
            # keep where col(tq-local) >= row(tk-local)
            diagmask = consts.tile([128, 128], mmdt)
            make_upper_triangular(nc, diagmask, val=1.0, diag=True)

            # weights: first e-chunk in its own DMA so the first matmul
            # can start as soon as ~32KB has landed; rest in one batch
            wqv_sb = consts.tile([128, N_ET, 128], mmdt)
            wk_sb = consts.tile([128, N_ET, H], mmdt)
            for lo, hi in ((0, 1), (1, N_ET)):
                nc.scalar.dma_start(
                    out=wqv_sb[:, lo:hi, :],
                    in_=wqv_d[lo * 128 : hi * 128, :].rearrange(
                        "(et p) m -> p et m", p=128
                    ),
                )
                nc.scalar.dma_start(
                    out=wk_sb[:, lo:hi, :],
                    in_=wk_d[lo * 128 : hi * 128, :].rearrange(
                        "(et p) m -> p et m", p=128
                    ),
                )
            km_sb = consts.tile([128, N_KT], f32)
            nc.scalar.dma_start(
                out=km_sb, in_=km_d[:].rearrange("(kt p) -> p kt", p=128)
            )

            def load_xt(tb, first=False):
                xt = xt_pool.tile([128, N_ET, NQ], mmdt, tag="xt")
                tsl = bass.ts(tb, NQ)
                if first:
                    # et chunk 0 alone so the first matmul starts early
                    nc.sync.dma_start(
                        out=xt[:, 0:1, :],
                        in_=xT_d[0:128, tsl].rearrange(
                            "(et p) m -> p et m", p=128
                        ),
                    )
                    nc.sync.dma_start(
                        out=xt[:, 1:N_ET, :],
                        in_=xT_d[128:E, tsl].rearrange(
                            "(et p) m -> p et m", p=128
                        ),
                    )
                else:
                    nc.sync.dma_start(
                        out=xt[:, :, :],
                        in_=xT_d[:, tsl].rearrange("(et p) m -> p et m", p=128),
                    )
                return xt

            xt0 = load_xt(0, first=True)
            xt1 = load_xt(1)

            qvT = big.tile([128, T], mmdt)  # rows 0:64 q^T, rows 64:128 v^T
            qd = big.tile([128, T], mmdt)   # rows 64:128 = q^T dup (top unused)
            # kST: col-block j holds k-tile 2j on rows 0:64, 2j+1 on 64:128
            kst = big.tile([128, T // 2], mmdt)
            vA = big.tile([128, N_KT, H + 1], mmdt)  # v natural + ones col
            nc.vector.memset(vA[:, :, H : H + 1], 1.0)

            def project_qv(tb, xt):
                """[Wq|Wv] pass for superblock tb; fills qvT/qd cols, vA."""
                tsl = bass.ts(tb, NQ)
                qv_ps = ps_proj.tile([128, NQ], f32, tag="proj")
                for et in range(N_ET):
                    nc.tensor.matmul(
                        qv_ps,
                        lhsT=wqv_sb[:, et, :],
                        rhs=xt[:, et, :],
                        start=(et == 0),
                        stop=(et == N_ET - 1),
                    )
                nc.vector.tensor_copy(qvT[:, tsl], qv_ps)
                # duplicate q onto partitions 64:128 for row-tiled scores
                nc.vector.tensor_copy(qd[64:128, tsl], qv_ps[0:64, :])

                # v natural tiles; padding mask folded into [v | ones] rows
                for kt in range(4 * tb, 4 * tb + 4):
                    vtr = ps_proj.tile([128, H], mmdt, tag="proj")
                    nc.tensor.transpose(
                        vtr,
                        qvT[64:128, kt * 128 : (kt + 1) * 128],
                        ident16[64:128, 64:128],
                    )
                    nc.vector.tensor_scalar_mul(
                        vA[:, kt, 0:H], vtr, km_sb[:, kt : kt + 1]
                    )
                    nc.gpsimd.tensor_scalar_mul(
                        vA[:, kt, H : H + 1],
                        vA[:, kt, H : H + 1],
                        km_sb[:, kt : kt + 1],
                    )

            def project_k_pair(tb0, xt_a, xt_b):
                """Col-tiled k for superblocks (tb0, tb0+1) in one pass set."""
                k_ps = ps_proj.tile([128, NQ], f32, tag="proj")
                for et in range(N_ET):
                    st, sp = (et == 0), (et == N_ET - 1)
                    nc.tensor.matmul(
                        k_ps[0:64, :],
                        lhsT=wk_sb[:, et, :],
                        rhs=xt_a[:, et, :],
                        start=st,
                        stop=sp,
                    )
                    nc.tensor.matmul(
                        k_ps[64:128, :],
                        lhsT=wk_sb[:, et, :],
                        rhs=xt_b[:, et, :],
                        start=st,
                        stop=sp,
                        skip_group_check=True,
                    )
                # scatter into kST: 4 key-tiles per sb; even->top, odd->bottom
                for tb, half in ((tb0, 0), (tb0 + 1, 64)):
                    src = k_ps[half : half + 64, :]
                    for i in range(4):
                        kt = 4 * tb + i
                        blk = kt // 2
                        dst_half = (kt % 2) * 64
                        nc.vector.tensor_copy(
                            kst[dst_half : dst_half + 64,
                                blk * 128 : (blk + 1) * 128],
                            src[:, i * 128 : (i + 1) * 128],
                        )

            def attention(qsb):
                """Causal attention for query superblock qsb."""
                q0 = qsb * NQ
                kt_last = 4 * qsb + 3
                ot_ps = ps_ot.tile([H + 1, NQ], f32, tag="ot")
                for kp in range((kt_last + 1) // 2):
                    kt0, kt1 = 2 * kp, 2 * kp + 1
                    c00 = max(0, 128 * kt0 - q0)
                    c01 = max(0, 128 * kt1 - q0)
                    stg = ps_st.tile([128, 2, NQ], f32, tag="st")
                    pt = pt_pool.tile([128, 2, NQ], mmdt, tag="pt")
                    ksl = bass.ts(kp, 128)
                    # two K=64 row-tiled matmuls, concurrent on the PE
                    nc.tensor.matmul(
                        stg[:, 0, c00:],
                        lhsT=kst[0:64, ksl],
                        rhs=qvT[0:64, q0 + c00 : q0 + NQ],
                        start=True,
                        stop=True,
                    )
                    nc.tensor.matmul(
                        stg[:, 1, c01:],
                        lhsT=kst[64:128, ksl],
                        rhs=qd[64:128, q0 + c01 : q0 + NQ],
                        start=True,
                        stop=True,
                    )
                    if kt1 < 4 * qsb:  # both sub-diagonal: one merged exp
                        nc.scalar.activation(
                            pt,
                            stg,
                            mybir.ActivationFunctionType.Exp,
                            scale=SCALE,
                        )
                    else:
                        for j, (kt, c0) in enumerate(((kt0, c00), (kt1, c01))):
                            nc.scalar.activation(
                                pt[:, j, c0:],
                                stg[:, j, c0:],
                                mybir.ActivationFunctionType.Exp,
                                scale=SCALE,
                            )
                    for j, (kt, c0) in enumerate(((kt0, c00), (kt1, c01))):
                        if kt >= 4 * qsb:  # diagonal-straddling tile
                            nc.vector.tensor_mul(
                                pt[:, j, c0 : c0 + 128],
                                pt[:, j, c0 : c0 + 128],
                                diagmask,
                            )
                        nc.tensor.matmul(
                            ot_ps[:, c0:],
                            lhsT=vA[:, kt, :],
                            rhs=pt[:, j, c0:],
                            start=(kt == 0),
                            stop=(kt == kt_last),
                        )

                otsb = otsb_pool.tile([H + 1, NQ], mmdt, tag="otsb")
                nc.vector.tensor_copy(otsb, ot_ps)
                osb = osb_pool.tile([128, NQ // 128, H], f32, tag="osb")
                for s in range(NQ // 128):
                    ott = ps_ot.tile([128, H + 1], mmdt, tag="ot")
                    nc.tensor.transpose(
                        ott,
                        otsb[:, s * 128 : (s + 1) * 128],
                        ident16[0 : H + 1, 0 : H + 1],
                    )
                    rec = small_pool.tile([128, 1], f32, tag="rec")
                    nc.vector.reciprocal(rec, ott[:, H : H + 1])
                    nc.vector.tensor_scalar_mul(osb[:, s, :], ott[:, 0:H], rec)
                return osb

            def write_out(qsb, osb, last):
                q0 = qsb * NQ
                out_eng = nc.sync if last else nc.gpsimd
                out_eng.dma_start(
                    out=out_d[q0 : q0 + NQ, :].rearrange(
                        "(s p) h -> p s h", p=128
                    ),
                    in_=osb,
                )

            # ---- pipelined over superblock pairs ----
            n_pairs = (N_QSB // 2) * repeat
            xts = [xt0, xt1]
            for pr in range(n_pairs):
                tb0 = (2 * pr) % N_QSB
                xt_a, xt_b = xts
                if pr + 1 < n_pairs:
                    nxt0 = ((2 * pr) + 2) % N_QSB
                    xts = [load_xt(nxt0), load_xt(nxt0 + 1)]
                project_qv(tb0, xt_a)
                project_k_pair(tb0, xt_a, xt_b)
                osb_a = attention(tb0)
                write_out(tb0, osb_a, last=False)
                project_qv(tb0 + 1, xt_b)
                osb_b = attention(tb0 + 1)
                write_out(tb0 + 1, osb_b, last=(pr == n_pairs - 1))

    nc.finalize()
    return nc


def get_nc(repeat=1):
    key = ("nc", repeat)
    if key not in _CACHE:
        _CACHE[key] = _build(repeat)
    return _CACHE[key]


def make_in_maps(x, Wq, Wk, Wv, key_padding_mask):
    np_dt = np.float16 if MM_DT == mybir.dt.float16 else np.float32
    x = np.asarray(x, dtype=np.float32)
    wqv = np.ascontiguousarray(
        np.concatenate([np.asarray(Wq), np.asarray(Wv)], axis=1), dtype=np_dt
    )
    wk = np.ascontiguousarray(np.asarray(Wk), dtype=np_dt)
    kmask = np.asarray(key_padding_mask).astype(np.float32)
    xT = np.ascontiguousarray(x.transpose(0, 2, 1).astype(np_dt))  # [B, E, T]
    return [
        {"xT": xT[b], "wqv": wqv, "wk": wk, "kmask": kmask[b]} for b in range(B)
    ]


def kernel(x, Wq, Wk, Wv, key_padding_mask, _trace=False, _trace_cores=None,
           _repeat=1):
    nc = get_nc(_repeat)
    in_maps = make_in_maps(x, Wq, Wk, Wv, key_padding_mask)
    res = run_bass_kernel_spmd(
        nc,
        in_maps,
        core_ids=list(range(B)),
        trace=_trace,
        trace_cores=_trace_cores,
    )
    _CACHE["last_results"] = res
    return np.stack([res.results[b]["out"] for b in range(B)], axis=0)
